# revision 1
# baseline (speedup 1.0000x reference)
"""Causal self-attention (B=1, T=4096, C=768, H=12, D=64) on 8 TRN2 NeuronCores.

Sharding: 8 cores = 4 head-groups (3 heads each) x 2 sequence-groups.
Core c: heads [3*hg, 3*hg+2] where hg=c//2; handles q-chunks of 256 rows,
global chunk g = 2*j + s (s=c%2, j=0..7) -- interleaving balances the causal
triangle so every core runs an identical instruction stream (SPMD), with the
boundary masks supplied as per-core data.

Precision strategy (PE fp32 matmul is 4 cyc/row; fp16 is 1): every matmul
runs in fp16 single-pass (10-bit mantissa ~ tf32-class given fp32 PSUM
accumulation; all operands here are O(1) so fp16 range is ample).
Flash-style attention in S^T = K@Q^T orientation: the softmax denominator
arrives free as the ones-column row of (V' P^T); no max-subtraction needed
since logits are O(5).  Causality = block skipping + 0/1 boundary masks.
Host sums the 4 head-group partial projections.
"""
import numpy as np

T, C, H, D = 4096, 768, 12, 64
NH = 3          # heads per core
QC = 256        # q rows per slot
P = 128

_nc_cache = {}


def split_multi_waits(nc):
    """Walrus here accepts only one sync wait per instruction: hoist extras
    onto standalone InstEventSemaphore instructions on the same engine."""
    import concourse.mybir as mybir
    n_split = 0
    for f in nc.m.functions:
        for bb in f.blocks:
            new_insts = []
            for inst in bb.instructions:
                si = inst.sync_info
                if si is not None and len(si.on_wait) > 1:
                    for w in si.on_wait[:-1]:
                        nop = mybir.InstEventSemaphore(
                            name=nc.get_next_instruction_name(), ins=[], outs=[])
                        nop.engine = inst.engine
                        nop.sync_info = mybir.SyncInfo(on_wait=[w], on_update=[])
                        nc.register_instruction(nop)
                        new_insts.append(nop)
                        n_split += 1
                    si.on_wait = si.on_wait[-1:]
                new_insts.append(inst)
            bb.instructions[:] = new_insts
    return n_split


def build_nc(Tloc=T):
    import concourse.bass as bass
    import concourse.mybir as mybir
    import concourse.tile as tile
    from concourse.masks import make_identity
    from contextlib import ExitStack

    f32r = mybir.dt.float32r
    f32 = mybir.dt.float32
    bf16 = mybir.dt.bfloat16
    EXP = mybir.ActivationFunctionType.Exp
    CPY = mybir.ActivationFunctionType.Copy
    ADD = mybir.AluOpType.add

    nslot = Tloc // (2 * QC)     # q-chunks per core
    nkb = Tloc // P              # k 128-blocks
    tq = nslot * QC              # q rows per core

    nc = bass.Bass(trn_type="TRN2")
    f16 = mybir.dt.float16
    xt16 = nc.dram_tensor("xt16", [C, Tloc], f16, kind="ExternalInput")
    xtq16 = nc.dram_tensor("xtq16", [C, tq], f16, kind="ExternalInput")
    wkv16 = nc.dram_tensor("wkv16", [C, 6 * D], f16, kind="ExternalInput")
    wq16 = nc.dram_tensor("wq16", [C, 2 * NH * D], f16, kind="ExternalInput")
    wpj16 = nc.dram_tensor("wpj16", [NH, D, C], f16, kind="ExternalInput")
    bias = nc.dram_tensor("bias", [P, 6], f32, kind="ExternalInput")
    mask = nc.dram_tensor("mask", [nslot, P, 1024], f16, kind="ExternalInput")
    out = nc.dram_tensor("out", [tq, C], f32, kind="ExternalOutput")

    # m-chunk -> (kind, head) for the packed [k0 k1 k2 v0 v1 v2] KV weights
    kv_map = [[("K", 0), ("K", 1)], [("K", 2), ("V", 0)], [("V", 1), ("V", 2)]]

    with tile.TileContext(nc) as tc, ExitStack() as ctx:
        singles = ctx.enter_context(tc.tile_pool(name="singles", bufs=1))
        xthp = ctx.enter_context(tc.tile_pool(name="xth", bufs=2))
        xqp = ctx.enter_context(tc.tile_pool(name="xq", bufs=2))
        vstp = ctx.enter_context(tc.tile_pool(name="vst", bufs=2))
        qtp = ctx.enter_context(tc.tile_pool(name="qt", bufs=2))
        mtp = ctx.enter_context(tc.tile_pool(name="mt", bufs=2))
        ptp = ctx.enter_context(tc.tile_pool(name="pt", bufs=3))
        rbp = ctx.enter_context(tc.tile_pool(name="rb", bufs=2))
        ytp = ctx.enter_context(tc.tile_pool(name="yt", bufs=2))
        ostp = ctx.enter_context(tc.tile_pool(name="ost", bufs=2))
        psg = ctx.enter_context(tc.tile_pool(name="psg", bufs=2, space="PSUM"))
        psy = ctx.enter_context(tc.tile_pool(name="psy", bufs=2, space="PSUM"))
        psm = ctx.enter_context(tc.tile_pool(name="psm", bufs=2, space="PSUM"))

        ident_f = singles.tile([64, 64], f32)
        make_identity(nc, ident_f)
        ones_f = singles.tile([1, 64], f32)
        nc.vector.memset(ones_f, 1.0)
        ones64 = singles.tile([1, 64], f32r)
        nc.vector.tensor_copy(ones64, ones_f)
        onesk_f = singles.tile([P, 32], f16)
        nc.vector.memset(onesk_f, 1.0)

        wkv_t = singles.tile([P, 6, 6 * D], f16)
        wq_t = singles.tile([P, 6, 2 * NH * D], f16)
        for c in range(6):
            nc.sync.dma_start(wkv_t[:, c], wkv16[P * c:P * c + P, :])
            nc.sync.dma_start(wq_t[:, c], wq16[P * c:P * c + P, :])
        wpj_t = []
        for h in range(NH):
            w1 = singles.tile([64, C], f16, tag=f"wpj{h}", name=f"wpj{h}")
            nc.sync.dma_start(w1, wpj16[h])
            wpj_t.append(w1)
        b_t = singles.tile([P, 6], f32)
        nc.sync.dma_start(b_t, bias[:, :])

        kt_t = [singles.tile([P, Tloc], f16, tag=f"kt{h}", name=f"kt{h}")
                for h in range(NH)]
        # bottom partition half starts zeroed; odd 64-col halves get K^T rows
        # DMA'd down, then the top copies of those columns are zeroed, giving
        # diag(K^T even-half, K^T odd-half) per 128-col block -> K=128 matmuls
        for h in range(NH):
            nc.vector.memset(kt_t[h][64:P, :], 0.0)
        vp_t = [singles.tile([P, nkb, 65], f16, tag=f"vp{h}", name=f"vp{h}")
                for h in range(NH)]
        for h in range(NH):
            nc.vector.tensor_copy(vp_t[h][:, :, 64], onesk_f[:, :nkb])

        # ---- Phase 1: K^T (bf16), V hi/lo from x^T, in 4 column-quarters ----
        QT4 = Tloc // 4
        for quarter in range(4):
            xh = [xthp.tile([P, QT4], f16, tag=f"xh{c}", name=f"xh{c}")
                  for c in range(6)]
            for c in range(6):
                nc.sync.dma_start(
                    xh[c], xt16[P * c:P * c + P, QT4 * quarter:QT4 * (quarter + 1)])
            for m in range(3):
                for n0 in range(0, QT4, 512):
                    w = min(512, QT4 - n0)
                    ps = psg.tile([P, 512], f32, tag="sg", name="ps")[:, :w]
                    for c in range(6):
                        nc.tensor.matmul(ps, wkv_t[:, c, P * m:P * m + P],
                                         xh[c][:, n0:n0 + w],
                                         start=(c == 0), stop=(c == 5))
                    g0 = QT4 * quarter + n0
                    for sub, (kind, h) in enumerate(kv_map[m]):
                        rows = slice(64 * sub, 64 * sub + 64)
                        bsl = b_t[rows, m:m + 1]
                        if kind == "K":
                            nc.vector.tensor_scalar(
                                kt_t[h][0:64, g0:g0 + w], ps[rows], bsl, None, ADD)
                        else:
                            vst = vstp.tile([64, 512], f32, tag="vst",
                                            name="vst")[:, :w]
                            nc.vector.tensor_scalar(vst, ps[rows], bsl, None, ADD)
                            for i in range(w // P):
                                tp = psm.tile([P, 64], f32, tag="psm", name="tp")
                                nc.tensor.transpose(
                                    tp, vst[:, P * i:P * (i + 1)], ident_f)
                                blk = g0 // P + i
                                nc.scalar.activation(
                                    vp_t[h][:, blk, 0:64], tp, CPY)

        for h in range(NH):
            odd_top = kt_t[h][0:64].rearrange("p (b t) -> p b t", t=P)[:, :, 64:P]
            odd_bot = kt_t[h][64:P].rearrange("p (b t) -> p b t", t=P)[:, :, 64:P]
            nc.sync.dma_start(odd_bot, odd_top)
            nc.vector.memset(odd_top, 0.0)

        # ---- Phase 2: per q-slot: Q^T, attention, projection ----
        for j in range(nslot):
            xq = xqp.tile([P, 6, QC], f16, tag="xq", name="xq")
            for c in range(6):
                nc.sync.dma_start(xq[:, c], xtq16[P * c:P * c + P, QC * j:QC * (j + 1)])
            qt_t = []
            for h in range(NH):
                psq = psm.tile([P, QC], f32, tag="psm", name="psq")
                for c in range(6):
                    nc.tensor.matmul(psq, wq_t[:, c, P * h:P * (h + 1)], xq[:, c],
                                     start=(c == 0), stop=(c == 5))
                qh = qtp.tile([P, QC], f16, tag=f"qt{h}", name=f"qt{h}")
                nc.vector.tensor_scalar(qh, psq, b_t[:, 3 + h:4 + h], None, ADD)
                qt_t.append(qh)

            mt = mtp.tile([P, 1024], f16, tag="mt", name="mt")
            nc.sync.dma_start(mt, mask[j])

            yt_t = []
            for h in range(NH):
                yacc = psy.tile([65, QC], f32, tag="yacc", name="yacc")

                def s_group(g):
                    sg = psg.tile([P, 1024], f32, tag="sg", name="sg")
                    for i in range(4):
                        kb = 4 * g + i
                        nc.tensor.matmul(sg[:, QC * i:QC * (i + 1)],
                                         kt_t[h][:, P * kb:P * (kb + 1)],
                                         qt_t[h], start=True, stop=True)
                    return sg

                # software pipeline: issue S(g+1) before PV(g) so the PE has
                # work while ACT runs exp(g)
                sg_cur = s_group(0)
                for g in range(j + 1):
                    sg_next = s_group(g + 1) if g < j else None
                    pt = ptp.tile([P, 1024], f16, tag="pt", name="pt")
                    nc.scalar.activation(pt, sg_cur, EXP, scale=0.125)
                    if g == j:
                        nc.vector.tensor_mul(pt, pt, mt)
                    for i in range(4):
                        kb = 4 * g + i
                        nc.tensor.matmul(yacc, vp_t[h][:, kb],
                                         pt[:, QC * i:QC * (i + 1)],
                                         start=(g == 0 and i == 0),
                                         stop=(g == j and i == 3))
                    sg_cur = sg_next
                # normalize: y^T = y'[0:64] / y'[64]; replicate the denominator
                # row via a K=1 ones matmul, reciprocal on all 64 partitions
                den = rbp.tile([1, QC], f32r, tag="den", name="den")
                nc.vector.tensor_copy(den, yacc[64:65])
                bc = psm.tile([64, QC], f32, tag="psm", name="bc")
                nc.tensor.matmul(bc, ones64, den, start=True, stop=True)
                rb = rbp.tile([64, QC], f32, tag="rb", name="rb")
                nc.vector.reciprocal(rb, bc)
                yt = ytp.tile([64, QC], f16, tag=f"yt{h}", name=f"yt{h}")
                nc.vector.tensor_mul(yt, yacc[0:64], rb)
                yt_t.append(yt)

            ost = ostp.tile([P, 2, C], f32, tag="ost", name="ost")
            for qb in range(2):
                for (n0, nw) in [(0, 512), (512, 256)]:
                    pp = psm.tile([P, nw], f32, tag="psm", name="pp")
                    for h in range(NH):
                        nc.tensor.matmul(pp, yt_t[h][:, P * qb:P * (qb + 1)],
                                         wpj_t[h][:, n0:n0 + nw],
                                         start=(h == 0), stop=(h == NH - 1))
                    nc.vector.tensor_copy(ost[:, qb, n0:n0 + nw], pp)
            for qb in range(2):
                nc.sync.dma_start(
                    out[QC * j + P * qb:QC * j + P * (qb + 1), :], ost[:, qb])

    split_multi_waits(nc)
    return nc


def make_in_maps(x, W_qkv, b_qkv, W_proj, Tloc=T):
    """Shard the full inputs into the 8 per-core input maps."""
    nslot = Tloc // (2 * QC)
    xT = np.ascontiguousarray(x.reshape(Tloc, C).T).astype(np.float32)
    xT16 = xT.astype(np.float16)

    kk = np.arange(P)
    qq = np.arange(QC)
    in_maps = []
    for core in range(8):
        hg, s = core // 2, core % 2
        heads = [3 * hg + i for i in range(NH)]
        wk = [W_qkv[:, C + 64 * h:C + 64 * h + 64] for h in heads]
        wv = [W_qkv[:, 2 * C + 64 * h:2 * C + 64 * h + 64] for h in heads]
        wkv_c = np.concatenate(wk + wv, axis=1)
        wq_c = np.concatenate(
            [np.tile(W_qkv[:, 64 * h:64 * h + 64], (1, 2)) for h in heads], axis=1)
        wpj_c = np.stack([W_proj[64 * h:64 * h + 64, :] for h in heads])
        wkv_16 = np.ascontiguousarray(wkv_c).astype(np.float16)
        wq_16 = np.ascontiguousarray(wq_c).astype(np.float16)
        wpj_16 = np.ascontiguousarray(wpj_c).astype(np.float16)

        bk = [b_qkv[C + 64 * h:C + 64 * h + 64] for h in heads]
        bv = [b_qkv[2 * C + 64 * h:2 * C + 64 * h + 64] for h in heads]
        bkv_c = np.concatenate(bk + bv)          # [384]
        bq_c = np.concatenate([b_qkv[64 * h:64 * h + 64] for h in heads])  # [192]
        bias_c = np.zeros((P, 6), np.float32)
        bias_c[:, 0:3] = bkv_c.reshape(3, P).T
        for hi_, h in enumerate(heads):
            bias_c[0:64, 3 + hi_] = b_qkv[64 * h:64 * h + 64]
            bias_c[64:P, 3 + hi_] = b_qkv[64 * h:64 * h + 64]

        qcols = np.concatenate(
            [np.arange(QC * (2 * j + s), QC * (2 * j + s) + QC) for j in range(nslot)])
        xtq_16 = np.ascontiguousarray(xT16[:, qcols])

        mask_c = np.zeros((nslot, P, 1024), np.float32)
        for j in range(nslot):
            q0 = QC * (2 * j + s)
            for i in range(4):
                k0 = P * (4 * j + i)
                mask_c[j, :, QC * i:QC * (i + 1)] = (
                    (k0 + kk[:, None]) <= (q0 + qq[None, :]))

        in_maps.append({
            "xt16": xT16, "xtq16": xtq_16,
            "wkv16": wkv_16, "wq16": wq_16, "wpj16": wpj_16,
            "bias": bias_c, "mask": mask_c.astype(np.float16),
        })
    return in_maps


def unshard(results, b_proj, Tloc=T):
    nslot = Tloc // (2 * QC)
    out = np.zeros((Tloc, C), np.float64)
    for core in range(8):
        s = core % 2
        r = results[core]["out"].astype(np.float64)
        for j in range(nslot):
            g0 = QC * (2 * j + s)
            out[g0:g0 + QC] += r[QC * j:QC * (j + 1)]
    out += b_proj.astype(np.float64)
    return out.astype(np.float32).reshape(1, Tloc, C)


_last_result = {}


def kernel(x, mask, W_qkv, b_qkv, W_proj, b_proj):
    from concourse.bass_utils import run_bass_kernel_spmd
    x = np.asarray(x, np.float32)
    W_qkv = np.asarray(W_qkv, np.float32)
    b_qkv = np.asarray(b_qkv, np.float32)
    W_proj = np.asarray(W_proj, np.float32)
    b_proj = np.asarray(b_proj, np.float32)

    if "nc" not in _nc_cache:
        _nc_cache["nc"] = build_nc(T)
    nc = _nc_cache["nc"]
    in_maps = make_in_maps(x, W_qkv, b_qkv, W_proj, T)
    import os
    kwargs = {}
    if os.environ.get("BASS_KERNEL_TRACE"):
        kwargs = dict(trace=True, trace_cores=list(range(8)))
    res = run_bass_kernel_spmd(nc, in_maps, core_ids=list(range(8)), **kwargs)
    _last_result["res"] = res
    return unshard([r for r in res.results], b_proj, T)

